# revision 18
# baseline (speedup 1.0000x reference)
"""Trainium2 Bass kernel for nn_Attention_53463752901338.

Computes K = rope(x @ Wk.T + bk), Q = rope(x @ Wq.T + bq), V = x @ Wv.T + bv
with x (16, 1024, 2048), W* (2048, 2048), b* (2048,).

Strategy: data-parallel over batch — each of the 8 NeuronCores gets 2 of the
16 batches (2048 tokens) and all three weight matrices; no collectives.
Matmuls run in bf16 (fp32 accumulate in PSUM); RoPE + bias run in fp32 on the
vector/scalar engines fused into the PSUM->SBUF evacuation.

Host-side prep (untimed): x is transposed to (d, tok) per core and cast bf16;
W for K/Q is row-permuted so the even/odd feature deinterleave of RoPE becomes
two contiguous halves; weights are laid out per (chunk-pair, k-group) so each
phase's weights are one contiguous DMA.
"""

import sys

if "/opt/trn_rl_repo" not in sys.path:
    sys.path.insert(0, "/opt/trn_rl_repo")

import numpy as np
import ml_dtypes

import concourse.bass as bass
import concourse.mybir as mybir
import concourse.tile as tile
from concourse import bacc
from concourse.bass_utils import run_bass_kernel_spmd

B, S, D = 16, 1024, 2048
N_CORES = 8
TOK = B * S // N_CORES          # 2048 tokens per core
KT = D // 128                   # 16 contraction tiles
NT = TOK // 128                 # 16 token tiles per core
BF16 = mybir.dt.bfloat16
F32 = mybir.dt.float32
NPBF16 = ml_dtypes.bfloat16

_COMPILED = None


def _build():
    nc = bacc.Bacc("TRN2", target_bir_lowering=False, debug=False,
                   num_devices=N_CORES)

    xT_d = nc.dram_tensor("xT", (D, TOK), BF16, kind="ExternalInput")
    w_d = {p: nc.dram_tensor(f"W{p}", (2, 4, 128, 4096), BF16,
                             kind="ExternalInput") for p in "KQV"}
    b_d = {p: nc.dram_tensor(f"b{p}", (128, D), F32, kind="ExternalInput")
           for p in "KQV"}
    cos_d = nc.dram_tensor("cos", (128, 8), F32, kind="ExternalInput")
    sin_d = nc.dram_tensor("sin", (128, 8), F32, kind="ExternalInput")
    # outputs viewed as (tok, half, 1024): half 0 = cols 0:1024, half 1 = 1024:2048
    o_d = {p: nc.dram_tensor(f"O{p}", (TOK, 2, 1024), F32,
                             kind="ExternalOutput") for p in "KQV"}

    MULT = mybir.AluOpType.mult
    ADD = mybir.AluOpType.add
    SUB = mybir.AluOpType.subtract

    with tile.TileContext(nc) as tc:
        with (
            tc.tile_pool(name="xp", bufs=1) as xp,
            tc.tile_pool(name="wp", bufs=8) as wp,
            tc.tile_pool(name="cp", bufs=1) as cp,
            tc.tile_pool(name="ep", bufs=3) as ep,
            tc.tile_pool(name="svp", bufs=1) as svp,
            tc.tile_pool(name="pp", bufs=4, space=bass.MemorySpace.PSUM) as pp,
        ):
            # Issue order drives Tile's DMA priority: x tiles and the first
            # phase's weights must land first; cos/sin/bias follow; later
            # phases' biases load lazily inside their phase.
            # DMA issue order matches the startup block's k-major consumption
            # order: each W k-group lands right before the 4 xT k-tiles that
            # stream against it.
            xt = []
            w0 = []
            for k in range(KT):
                t_ = xp.tile([128, TOK], BF16, tag=f"x{k}", name=f"xt{k}")
                nc.sync.dma_start(t_[:], xT_d.ap()[k * 128:(k + 1) * 128, :])
                xt.append(t_)
                if k % 4 == 0:
                    w_ = wp.tile([128, 4096], BF16, tag="w", name="w0")
                    nc.sync.dma_start(w_[:], w_d["K"].ap()[0, k // 4])
                    w0.append(w_)

            cos_sb = cp.tile([128, 8], F32, tag="cos")
            nc.sync.dma_start(cos_sb[:], cos_d.ap()[:])
            sin_sb = cp.tile([128, 8], F32, tag="sin")
            nc.sync.dma_start(sin_sb[:], sin_d.ap()[:])
            bias_sb = {}

            for proj, pair in [("K", 0), ("K", 1), ("Q", 0), ("Q", 1),
                               ("V", 0), ("V", 1)]:
                if proj not in bias_sb:
                    bias_sb[proj] = cp.tile([128, D], F32, tag=f"b{proj}",
                                            name=f"bias{proj}")
                    nc.sync.dma_start(bias_sb[proj][:], b_d[proj].ap()[:])
                if proj == "K" and pair == 0:
                    wt = w0
                else:
                    wt = []
                    for g in range(4):
                        w_ = wp.tile([128, 4096], BF16, tag="w")
                        nc.sync.dma_start(w_[:], w_d[proj].ap()[pair, g])
                        wt.append(w_)

                be = bias_sb[proj][:, pair * 512:(pair + 1) * 512]
                bo = bias_sb[proj][:, 1024 + pair * 512:1024 + (pair + 1) * 512]

                def mm_group(ps, t, wt=wt):
                    for k in range(KT):
                        g, kk = divmod(k, 4)
                        lhsT = xt[k][:, t * 128:(t + 1) * 128]
                        nc.tensor.matmul(
                            ps[:, 0:512], lhsT,
                            wt[g][:, kk * 1024:kk * 1024 + 512],
                            start=(k == 0), stop=(k == KT - 1))
                        nc.tensor.matmul(
                            ps[:, 512:1024], lhsT,
                            wt[g][:, kk * 1024 + 512:kk * 1024 + 1024],
                            start=(k == 0), stop=(k == KT - 1))

                def epilogue(ps, t, proj=proj, pair=pair, be=be, bo=bo):
                    out_t = ep.tile([128, 2, 512], F32, tag="out",
                                    name="out_t")
                    if proj == "V":
                        nc.vector.tensor_add(out_t[:, 0, :], ps[:, 0:512], be)
                        nc.vector.tensor_add(out_t[:, 1, :], ps[:, 512:1024],
                                             bo)
                    else:
                        st = t % 8
                        cos_ap = cos_sb[:, st:st + 1]
                        sin_ap = sin_sb[:, st:st + 1]
                        yeb = ep.tile([128, 512], F32, tag="yeb", name="yeb")
                        yob = ep.tile([128, 512], F32, tag="yob", name="yob")
                        u = ep.tile([128, 512], F32, tag="u", name="u")
                        v = ep.tile([128, 512], F32, tag="u", name="v")
                        nc.vector.tensor_add(yeb[:], ps[:, 0:512], be)
                        nc.vector.tensor_add(yob[:], ps[:, 512:1024], bo)
                        nc.scalar.mul(u[:], yob[:], sin_ap)
                        nc.vector.scalar_tensor_tensor(
                            out_t[:, 0, :], yeb[:], cos_ap, u[:], MULT, SUB)
                        nc.scalar.mul(v[:], yob[:], cos_ap)
                        nc.vector.scalar_tensor_tensor(
                            out_t[:, 1, :], yeb[:], sin_ap, v[:], MULT, ADD)

                    nc.sync.dma_start(
                        o_d[proj].ap()[t * 128:(t + 1) * 128, :,
                                       pair * 512:(pair + 1) * 512],
                        out_t[:])

                if proj == "K" and pair == 0:
                    # Startup: while the initial 12.4 MB x/W load streams in,
                    # the in-order PE stream must have work matched to DMA
                    # arrival order.  Process t=0..7 in two half-contraction
                    # passes: k-major blocks of 4 token tiles over k=0..7
                    # (only the first 6 MB of data), partial sums parked in
                    # SBUF as bf16 (bias folded in), then the k=8..15 halves
                    # merge via the epilogue's bias operand slot.
                    sv = {}
                    for blk in range(2):
                        psA = [pp.tile([128, 1024], F32, tag="ps",
                                       name=f"psA{blk}_{i}") for i in range(4)]
                        for k in range(8):
                            g, kk = divmod(k, 4)
                            for i, psi in enumerate(psA):
                                t = blk * 4 + i
                                lhsT = xt[k][:, t * 128:(t + 1) * 128]
                                nc.tensor.matmul(
                                    psi[:, 0:512], lhsT,
                                    wt[g][:, kk * 1024:kk * 1024 + 512],
                                    start=(k == 0), stop=(k == 7))
                                nc.tensor.matmul(
                                    psi[:, 512:1024], lhsT,
                                    wt[g][:, kk * 1024 + 512:kk * 1024 + 1024],
                                    start=(k == 0), stop=(k == 7))
                        for i, psi in enumerate(psA):
                            t = blk * 4 + i
                            s_ = svp.tile([128, 1024], BF16, tag=f"sv{t}",
                                          name=f"sv{t}")
                            nc.vector.tensor_add(s_[:, 0:512],
                                                 psi[:, 0:512], be)
                            nc.vector.tensor_add(s_[:, 512:1024],
                                                 psi[:, 512:1024], bo)
                            sv[t] = s_
                    for t in range(8):
                        ps = pp.tile([128, 1024], F32, tag="ps", name="psB")
                        for k in range(8, KT):
                            g, kk = divmod(k, 4)
                            lhsT = xt[k][:, t * 128:(t + 1) * 128]
                            nc.tensor.matmul(
                                ps[:, 0:512], lhsT,
                                wt[g][:, kk * 1024:kk * 1024 + 512],
                                start=(k == 8), stop=(k == KT - 1))
                            nc.tensor.matmul(
                                ps[:, 512:1024], lhsT,
                                wt[g][:, kk * 1024 + 512:kk * 1024 + 1024],
                                start=(k == 8), stop=(k == KT - 1))
                        epilogue(ps, t, be=sv[t][:, 0:512],
                                 bo=sv[t][:, 512:1024])
                    t_rest = range(8, NT)
                else:
                    t_rest = range(NT)

                for t in t_rest:
                    ps = pp.tile([128, 1024], F32, tag="ps", name="ps")
                    mm_group(ps, t)
                    epilogue(ps, t)

    nc.compile()
    return nc


def _get_compiled():
    global _COMPILED
    if _COMPILED is None:
        _COMPILED = _build()
    return _COMPILED


def _prep_weight(W, rope_perm):
    """(D, D) f32 nn.Linear weight -> (2, 4, 128, 4096) bf16 device layout.

    Output feature chunks c = fo//512; pair 0 holds chunks (0, 2), pair 1
    holds (1, 3), each k-group g holds k-tiles 4g..4g+3 laid out
    [partition][kk][512 e-cols, 512 o-cols... ] as [128, kk*1024 + c_half*512].
    """
    Wp = np.concatenate([W[0::2, :], W[1::2, :]], axis=0) if rope_perm else W
    WT = np.ascontiguousarray(Wp.T)                      # (d_in, fo)
    WTr = WT.reshape(KT, 128, 4, 512)                    # (k, row, chunk, col)
    pairs = np.stack([WTr[:, :, [0, 2], :], WTr[:, :, [1, 3], :]], axis=0)
    dev = pairs.reshape(2, KT, 128, 1024)                # (pair, k, row, 1024)
    dev = dev.reshape(2, 4, 4, 128, 1024).transpose(0, 1, 3, 2, 4)
    dev = np.ascontiguousarray(dev.reshape(2, 4, 128, 4096))
    return dev.astype(NPBF16)


def _prep_bias(b, rope_perm):
    bp = np.concatenate([b[0::2], b[1::2]]) if rope_perm else b
    return np.ascontiguousarray(
        np.broadcast_to(bp.astype(np.float32), (128, D)))


def _prep_inputs(x, Wk, bk, Wq, bq, Wv, bv):
    inv_freq = 1.0 / (10000.0 ** (
        np.arange(0.0, D, 2.0, dtype=np.float32) / np.float32(D)))
    freqs = inv_freq * np.arange(S, dtype=np.float32)
    cos = np.cos(freqs).astype(np.float32)               # (1024,)
    sin = np.sin(freqs).astype(np.float32)
    cos_t = np.ascontiguousarray(cos.reshape(8, 128).T)  # (128, 8)
    sin_t = np.ascontiguousarray(sin.reshape(8, 128).T)

    shared = {
        "WK": _prep_weight(Wk, True),
        "WQ": _prep_weight(Wq, True),
        "WV": _prep_weight(Wv, False),
        "bK": _prep_bias(bk, True),
        "bQ": _prep_bias(bq, True),
        "bV": _prep_bias(bv, False),
        "cos": cos_t,
        "sin": sin_t,
    }

    xall = np.asarray(x, dtype=np.float32).reshape(N_CORES, TOK, D)
    in_maps = []
    for c in range(N_CORES):
        xT = np.ascontiguousarray(xall[c].T).astype(NPBF16)   # (D, TOK)
        in_maps.append({"xT": xT, **shared})
    return in_maps


def _assemble(results):
    outs = []
    for name in ("OK", "OQ", "OV"):
        full = np.concatenate(
            [np.asarray(results[c][name], dtype=np.float32).reshape(TOK, D)
             for c in range(N_CORES)], axis=0)
        outs.append(full.reshape(B, S, D))
    # reference returns (K, Q, V)
    return tuple(outs)


def _run(inputs, **run_kwargs):
    nc = _get_compiled()
    in_maps = _prep_inputs(**{k: np.asarray(v) for k, v in inputs.items()})
    last_err = None
    for _attempt in range(3):
        try:
            res = run_bass_kernel_spmd(nc, in_maps,
                                       core_ids=list(range(N_CORES)),
                                       **run_kwargs)
            return _assemble(res.results), res
        except Exception as e:  # transient NRT device errors — retry
            last_err = e
            import time
            time.sleep(2.0)
    raise last_err


def kernel(**inputs):
    outputs, _ = _run(inputs)
    return outputs


# revision 19
# speedup vs baseline: 1.0217x; 1.0217x over previous
"""Trainium2 Bass kernel for nn_Attention_53463752901338.

Computes K = rope(x @ Wk.T + bk), Q = rope(x @ Wq.T + bq), V = x @ Wv.T + bv
with x (16, 1024, 2048), W* (2048, 2048), b* (2048,).

Strategy: data-parallel over batch — each of the 8 NeuronCores gets 2 of the
16 batches (2048 tokens) and all three weight matrices; no collectives.
Matmuls run in bf16 (fp32 accumulate in PSUM); RoPE + bias run in fp32 on the
vector/scalar engines fused into the PSUM->SBUF evacuation.

Host-side prep (untimed): x is transposed to (d, tok) per core and cast bf16;
W for K/Q is row-permuted so the even/odd feature deinterleave of RoPE becomes
two contiguous halves; weights are laid out per (chunk-pair, k-group) so each
phase's weights are one contiguous DMA.
"""

import sys

if "/opt/trn_rl_repo" not in sys.path:
    sys.path.insert(0, "/opt/trn_rl_repo")

import numpy as np
import ml_dtypes

import concourse.bass as bass
import concourse.mybir as mybir
import concourse.tile as tile
from concourse import bacc
from concourse.bass_utils import run_bass_kernel_spmd

B, S, D = 16, 1024, 2048
N_CORES = 8
TOK = B * S // N_CORES          # 2048 tokens per core
KT = D // 128                   # 16 contraction tiles
NT = TOK // 128                 # 16 token tiles per core
BF16 = mybir.dt.bfloat16
F32 = mybir.dt.float32
NPBF16 = ml_dtypes.bfloat16

_COMPILED = None


def _build():
    nc = bacc.Bacc("TRN2", target_bir_lowering=False, debug=False,
                   num_devices=N_CORES)

    xT_d = nc.dram_tensor("xT", (D, TOK), BF16, kind="ExternalInput")
    w_d = {p: nc.dram_tensor(f"W{p}", (2, 4, 128, 4096), BF16,
                             kind="ExternalInput") for p in "KQV"}
    b_d = {p: nc.dram_tensor(f"b{p}", (128, D), F32, kind="ExternalInput")
           for p in "KQV"}
    cos_d = nc.dram_tensor("cos", (128, 8), F32, kind="ExternalInput")
    sin_d = nc.dram_tensor("sin", (128, 8), F32, kind="ExternalInput")
    # outputs viewed as (tok, half, 1024): half 0 = cols 0:1024, half 1 = 1024:2048
    o_d = {p: nc.dram_tensor(f"O{p}", (TOK, 2, 1024), F32,
                             kind="ExternalOutput") for p in "KQV"}

    MULT = mybir.AluOpType.mult
    ADD = mybir.AluOpType.add
    SUB = mybir.AluOpType.subtract

    with tile.TileContext(nc) as tc:
        with (
            tc.tile_pool(name="xp", bufs=1) as xp,
            tc.tile_pool(name="wp", bufs=8) as wp,
            tc.tile_pool(name="cp", bufs=1) as cp,
            tc.tile_pool(name="ep", bufs=3) as ep,
            tc.tile_pool(name="svp", bufs=1) as svp,
            tc.tile_pool(name="pp", bufs=4, space=bass.MemorySpace.PSUM) as pp,
        ):
            # Issue order drives Tile's DMA priority: x tiles and the first
            # phase's weights must land first; cos/sin/bias follow; later
            # phases' biases load lazily inside their phase.
            # DMA issue order matches the startup block's k-major consumption
            # order: each W k-group lands right before the 4 xT k-tiles that
            # stream against it.
            cos_sb = cp.tile([128, 8], F32, tag="cos")
            nc.sync.dma_start(cos_sb[:], cos_d.ap()[:])
            sin_sb = cp.tile([128, 8], F32, tag="sin")
            nc.sync.dma_start(sin_sb[:], sin_d.ap()[:])
            bias_sb = {"K": cp.tile([128, D], F32, tag="bK", name="biasK")}
            nc.sync.dma_start(bias_sb["K"][:], b_d["K"].ap()[:])

            xt = []
            w0 = []
            for k in range(KT):
                t_ = xp.tile([128, TOK], BF16, tag=f"x{k}", name=f"xt{k}")
                nc.sync.dma_start(t_[:], xT_d.ap()[k * 128:(k + 1) * 128, :])
                xt.append(t_)
                if k % 4 == 0:
                    w_ = wp.tile([128, 4096], BF16, tag="w", name="w0")
                    nc.sync.dma_start(w_[:], w_d["K"].ap()[0, k // 4])
                    w0.append(w_)

            for proj, pair in [("K", 0), ("K", 1), ("Q", 0), ("Q", 1),
                               ("V", 0), ("V", 1)]:
                if proj not in bias_sb:
                    bias_sb[proj] = cp.tile([128, D], F32, tag=f"b{proj}",
                                            name=f"bias{proj}")
                    nc.sync.dma_start(bias_sb[proj][:], b_d[proj].ap()[:])
                if proj == "K" and pair == 0:
                    wt = w0
                else:
                    wt = []
                    for g in range(4):
                        w_ = wp.tile([128, 4096], BF16, tag="w")
                        nc.sync.dma_start(w_[:], w_d[proj].ap()[pair, g])
                        wt.append(w_)

                be = bias_sb[proj][:, pair * 512:(pair + 1) * 512]
                bo = bias_sb[proj][:, 1024 + pair * 512:1024 + (pair + 1) * 512]

                def mm_group(ps, t, wt=wt):
                    for k in range(KT):
                        g, kk = divmod(k, 4)
                        lhsT = xt[k][:, t * 128:(t + 1) * 128]
                        nc.tensor.matmul(
                            ps[:, 0:512], lhsT,
                            wt[g][:, kk * 1024:kk * 1024 + 512],
                            start=(k == 0), stop=(k == KT - 1))
                        nc.tensor.matmul(
                            ps[:, 512:1024], lhsT,
                            wt[g][:, kk * 1024 + 512:kk * 1024 + 1024],
                            start=(k == 0), stop=(k == KT - 1))

                def epilogue(ps, t, proj=proj, pair=pair, be=be, bo=bo):
                    out_t = ep.tile([128, 2, 512], F32, tag="out",
                                    name="out_t")
                    if proj == "V":
                        nc.vector.tensor_add(out_t[:, 0, :], ps[:, 0:512], be)
                        nc.vector.tensor_add(out_t[:, 1, :], ps[:, 512:1024],
                                             bo)
                    else:
                        st = t % 8
                        cos_ap = cos_sb[:, st:st + 1]
                        sin_ap = sin_sb[:, st:st + 1]
                        yeb = ep.tile([128, 512], F32, tag="yeb", name="yeb")
                        yob = ep.tile([128, 512], F32, tag="yob", name="yob")
                        u = ep.tile([128, 512], F32, tag="u", name="u")
                        v = ep.tile([128, 512], F32, tag="u", name="v")
                        nc.vector.tensor_add(yeb[:], ps[:, 0:512], be)
                        nc.vector.tensor_add(yob[:], ps[:, 512:1024], bo)
                        nc.scalar.mul(u[:], yob[:], sin_ap)
                        nc.vector.scalar_tensor_tensor(
                            out_t[:, 0, :], yeb[:], cos_ap, u[:], MULT, SUB)
                        nc.scalar.mul(v[:], yob[:], cos_ap)
                        nc.vector.scalar_tensor_tensor(
                            out_t[:, 1, :], yeb[:], sin_ap, v[:], MULT, ADD)

                    nc.sync.dma_start(
                        o_d[proj].ap()[t * 128:(t + 1) * 128, :,
                                       pair * 512:(pair + 1) * 512],
                        out_t[:])

                if proj == "K" and pair == 0:
                    # Startup: while the initial 12.4 MB x/W load streams in,
                    # the in-order PE stream must have work matched to DMA
                    # arrival order.  Process t=0..7 in two half-contraction
                    # passes: k-major blocks of 4 token tiles over k=0..7
                    # (only the first 6 MB of data), partial sums parked in
                    # SBUF as bf16 (bias folded in), then the k=8..15 halves
                    # merge via the epilogue's bias operand slot.
                    sv = {}
                    for blk in range(2):
                        psA = [pp.tile([128, 1024], F32, tag="ps",
                                       name=f"psA{blk}_{i}") for i in range(4)]
                        for k in range(8):
                            g, kk = divmod(k, 4)
                            for i, psi in enumerate(psA):
                                t = blk * 4 + i
                                lhsT = xt[k][:, t * 128:(t + 1) * 128]
                                nc.tensor.matmul(
                                    psi[:, 0:512], lhsT,
                                    wt[g][:, kk * 1024:kk * 1024 + 512],
                                    start=(k == 0), stop=(k == 7))
                                nc.tensor.matmul(
                                    psi[:, 512:1024], lhsT,
                                    wt[g][:, kk * 1024 + 512:kk * 1024 + 1024],
                                    start=(k == 0), stop=(k == 7))
                        for i, psi in enumerate(psA):
                            t = blk * 4 + i
                            s_ = svp.tile([128, 1024], BF16, tag=f"sv{t}",
                                          name=f"sv{t}")
                            nc.vector.tensor_add(s_[:, 0:512],
                                                 psi[:, 0:512], be)
                            nc.vector.tensor_add(s_[:, 512:1024],
                                                 psi[:, 512:1024], bo)
                            sv[t] = s_
                    for t in range(8):
                        ps = pp.tile([128, 1024], F32, tag="ps", name="psB")
                        for k in range(8, KT):
                            g, kk = divmod(k, 4)
                            lhsT = xt[k][:, t * 128:(t + 1) * 128]
                            nc.tensor.matmul(
                                ps[:, 0:512], lhsT,
                                wt[g][:, kk * 1024:kk * 1024 + 512],
                                start=(k == 8), stop=(k == KT - 1))
                            nc.tensor.matmul(
                                ps[:, 512:1024], lhsT,
                                wt[g][:, kk * 1024 + 512:kk * 1024 + 1024],
                                start=(k == 8), stop=(k == KT - 1))
                        epilogue(ps, t, be=sv[t][:, 0:512],
                                 bo=sv[t][:, 512:1024])
                    t_rest = range(8, NT)
                else:
                    t_rest = range(NT)

                for t in t_rest:
                    ps = pp.tile([128, 1024], F32, tag="ps", name="ps")
                    mm_group(ps, t)
                    epilogue(ps, t)

    nc.compile()
    return nc


def _get_compiled():
    global _COMPILED
    if _COMPILED is None:
        _COMPILED = _build()
    return _COMPILED


def _prep_weight(W, rope_perm):
    """(D, D) f32 nn.Linear weight -> (2, 4, 128, 4096) bf16 device layout.

    Output feature chunks c = fo//512; pair 0 holds chunks (0, 2), pair 1
    holds (1, 3), each k-group g holds k-tiles 4g..4g+3 laid out
    [partition][kk][512 e-cols, 512 o-cols... ] as [128, kk*1024 + c_half*512].
    """
    Wp = np.concatenate([W[0::2, :], W[1::2, :]], axis=0) if rope_perm else W
    WT = np.ascontiguousarray(Wp.T)                      # (d_in, fo)
    WTr = WT.reshape(KT, 128, 4, 512)                    # (k, row, chunk, col)
    pairs = np.stack([WTr[:, :, [0, 2], :], WTr[:, :, [1, 3], :]], axis=0)
    dev = pairs.reshape(2, KT, 128, 1024)                # (pair, k, row, 1024)
    dev = dev.reshape(2, 4, 4, 128, 1024).transpose(0, 1, 3, 2, 4)
    dev = np.ascontiguousarray(dev.reshape(2, 4, 128, 4096))
    return dev.astype(NPBF16)


def _prep_bias(b, rope_perm):
    bp = np.concatenate([b[0::2], b[1::2]]) if rope_perm else b
    return np.ascontiguousarray(
        np.broadcast_to(bp.astype(np.float32), (128, D)))


def _prep_inputs(x, Wk, bk, Wq, bq, Wv, bv):
    inv_freq = 1.0 / (10000.0 ** (
        np.arange(0.0, D, 2.0, dtype=np.float32) / np.float32(D)))
    freqs = inv_freq * np.arange(S, dtype=np.float32)
    cos = np.cos(freqs).astype(np.float32)               # (1024,)
    sin = np.sin(freqs).astype(np.float32)
    cos_t = np.ascontiguousarray(cos.reshape(8, 128).T)  # (128, 8)
    sin_t = np.ascontiguousarray(sin.reshape(8, 128).T)

    shared = {
        "WK": _prep_weight(Wk, True),
        "WQ": _prep_weight(Wq, True),
        "WV": _prep_weight(Wv, False),
        "bK": _prep_bias(bk, True),
        "bQ": _prep_bias(bq, True),
        "bV": _prep_bias(bv, False),
        "cos": cos_t,
        "sin": sin_t,
    }

    xall = np.asarray(x, dtype=np.float32).reshape(N_CORES, TOK, D)
    in_maps = []
    for c in range(N_CORES):
        xT = np.ascontiguousarray(xall[c].T).astype(NPBF16)   # (D, TOK)
        in_maps.append({"xT": xT, **shared})
    return in_maps


def _assemble(results):
    outs = []
    for name in ("OK", "OQ", "OV"):
        full = np.concatenate(
            [np.asarray(results[c][name], dtype=np.float32).reshape(TOK, D)
             for c in range(N_CORES)], axis=0)
        outs.append(full.reshape(B, S, D))
    # reference returns (K, Q, V)
    return tuple(outs)


def _run(inputs, **run_kwargs):
    nc = _get_compiled()
    in_maps = _prep_inputs(**{k: np.asarray(v) for k, v in inputs.items()})
    last_err = None
    for _attempt in range(3):
        try:
            res = run_bass_kernel_spmd(nc, in_maps,
                                       core_ids=list(range(N_CORES)),
                                       **run_kwargs)
            return _assemble(res.results), res
        except Exception as e:  # transient NRT device errors — retry
            last_err = e
            import time
            time.sleep(2.0)
    raise last_err


def kernel(**inputs):
    outputs, _ = _run(inputs)
    return outputs
